# revision 1
# baseline (speedup 1.0000x reference)
"""Position-attention (SAGAN-style) Bass kernel for 8 Trainium2 NeuronCores.

Reference computation (per batch b, with n = H*W = 4096 spatial tokens):
    q = Wq @ x + bq            [32, n]
    k = Wk @ x + bk            [32, n]
    v = Wv @ x + bv            [256, n]
    att = softmax_j(q_i . k_j) [n, n]
    out = gamma * (v @ att^T) + x

Sharding: 8 cores = 4 batches x 2 token-halves. Each core handles one batch's
2048-token half: it computes q only for its tokens i (scores^T[j, i] for all
j), so the softmax/exp work is split between the half-pair, while k/v are
computed redundantly (cheap). Output slices are disjoint -> no collectives.

Layout choices (all picked so the contraction dim lands on partitions):
  - scores^T[j, i] = lhsT(k[d, j_tile]) . rhs(q[d, i]): K=32 contraction.
    K=32 only uses 32 of 128 PE rows, so 4 j-tiles run concurrently in the
    four 32-row PE groups via tile_position; q/k are built 4x-replicated
    across partitions (host tiles Wq^T/Wk^T 4x along out-channels) so each
    row group finds its operands at base partition 32*t.
  - out[c, i] = sum_j v^T[j, c] * e^T[j, i]: K=128 per j-tile, accumulated
    over 32 j-tiles in PSUM. v^T[n_tile, c] comes directly from a matmul
    with lhsT = x (natural layout), rhs = Wv^T.
  - softmax needs no max-subtraction: scores std ~3.6, max ~22 << 88
    (fp32 exp overflow), so e = exp(scores) directly; rowsum[i] = sum_j
    e^T[j, i] is accumulated across j-tiles on DVE/GPSIMD and reduced over
    partitions with a ones-vector matmul; normalization (and gamma) fold
    into a per-i scale broadcast back to 128 partitions with a rank-1
    ones matmul.
Matmul operands are bf16 (fp32 PSUM accumulation); the residual `+ x` is
added in exact fp32.
"""

import os
import sys

for _p in (
    "/root/.axon_site",
    "/root/.axon_site/_ro/trn_rl_repo",
    "/root/.axon_site/_ro/pypackages",
    "/opt/trn_rl_repo",
):
    if os.path.isdir(_p) and _p not in sys.path:
        sys.path.append(_p)

import json

import numpy as np

from concourse import bass, mybir
from concourse.tile import TileContext

F32 = mybir.dt.float32
BF16 = mybir.dt.bfloat16
FP8 = mybir.dt.float8e4

# fp8 path for the big out-matmul: e and v in e4m3 with DoubleRow (2 fp8
# weights/PE cell -> K=256 per matmul). Scores are shifted by -EXP_SHIFT
# before exp so e fits e4m3's 448 max (max score ~22 for these stats); the
# shift cancels exactly between numerator and rowsum.
# Disabled: with a global shift, rows whose max score falls below
# shift-4.2 flush entirely to zero in e4m3 -> rowsum 0 -> inf/NaN. The
# usable shift window is too narrow to be robust; bf16 keeps full range.
USE_FP8 = False
EXP_SHIFT = 17.0

B, C, H, W = 4, 256, 64, 64
N = H * W            # 4096 tokens
NH = N // 2          # 2048 tokens per core (token half)
MID = C // 8         # 32 qk channels
JT = N // 128        # 32 j-tiles of 128 tokens
NBLK = NH // 512     # 4 i-blocks of 512 tokens per core


def _split_multi_waits(bir_bytes: bytes) -> bytes:
    """Workaround for this container's walrus: it accepts at most ONE sem-wait
    command per lowered instruction ('Too many sync wait commands'), while
    bass/Tile freely attach several. Split extra waits onto preceding NoOps
    on the same engine — per-engine program order makes this semantics-
    preserving (all waits still satisfied before the instruction runs)."""
    d = json.loads(bir_bytes)
    n_split = 0
    for f in d.get("functions", []):
        for bb in f.get("blocks", []):
            out = []
            for ins in bb.get("instructions", []):
                si = ins.get("sync_info")
                waits = si.get("on_wait") if si else None
                if waits and len(waits) > 1:
                    for w in waits[:-1]:
                        n_split += 1
                        out.append(
                            {
                                "debug": ins.get("debug", 0),
                                "engine": ins["engine"],
                                "ins": [],
                                "outs": [],
                                "name": f"{ins['name']}-ws{n_split}",
                                "opcode": "NoOp",
                                "sync_info": {"on_wait": [w], "on_update": []},
                            }
                        )
                    si["on_wait"] = [waits[-1]]
                out.append(ins)
            bb["instructions"] = out
    return json.dumps(d).encode()


_ws_applied = False


def _apply_wait_split_patch():
    global _ws_applied
    if _ws_applied:
        return
    _ws_applied = True
    from concourse import bass_utils, bass2jax

    orig = bass_utils.compile_bir_kernel

    def patched(bir_json, tmpdir, neff_name="file.neff"):
        return orig(_split_multi_waits(bytes(bir_json)), tmpdir, neff_name)

    bass_utils.compile_bir_kernel = patched
    bass2jax.compile_bir_kernel = patched


_apply_wait_split_patch()


def _build_program():
    nc = bass.Bass()

    xf_d = nc.declare_dram_parameter("xf", [C, N], F32, isOutput=False)
    xq_d = nc.declare_dram_parameter("xq", [C, NH], F32, isOutput=False)
    wT_d = nc.declare_dram_parameter("wT", [C, 512], F32, isOutput=False)
    bq4_d = nc.declare_dram_parameter("bq4", [128, 1], F32, isOutput=False)
    bk4_d = nc.declare_dram_parameter("bk4", [128, 1], F32, isOutput=False)
    bvP_d = nc.declare_dram_parameter("bvP", [128, 2], F32, isOutput=False)
    g128_d = nc.declare_dram_parameter("g128", [128, 1], F32, isOutput=False)
    out_d = nc.declare_dram_parameter("out", [C, NH], F32, isOutput=True)

    act = mybir.ActivationFunctionType

    with TileContext(nc) as tc:
        with (
            tc.tile_pool(name="const", bufs=1) as constp,
            tc.tile_pool(name="xqf", bufs=1) as xqfp,
            tc.tile_pool(name="xb", bufs=1) as xbp,
            tc.tile_pool(name="proj", bufs=1) as projp,
            tc.tile_pool(name="eblk", bufs=2) as eblkp,
            tc.tile_pool(name="small", bufs=4) as smallp,
            tc.tile_pool(name="res", bufs=4) as resp,
            tc.tile_pool(name="psA", bufs=2, space="PSUM") as psA,
            tc.tile_pool(name="psB", bufs=2, space="PSUM") as psB,
            tc.tile_pool(name="psR", bufs=2, space="PSUM") as psR,
        ):
            # ---- constants / weights ----
            w_f = constp.tile([128, 2, 512], F32, tag="wf")
            nc.sync.dma_start(out=w_f[:, 0, :], in_=wT_d[0:128, :])
            nc.sync.dma_start(out=w_f[:, 1, :], in_=wT_d[128:256, :])
            w_b = constp.tile([128, 2, 512], BF16, tag="wb")
            nc.vector.tensor_copy(w_b[:, :, :], w_f[:, :, :])

            bq4 = constp.tile([128, 1], F32, tag="bq4")
            nc.sync.dma_start(out=bq4[:, :], in_=bq4_d[:, :])
            bk4 = constp.tile([128, 1], F32, tag="bk4")
            nc.sync.dma_start(out=bk4[:, :], in_=bk4_d[:, :])
            g128 = constp.tile([128, 1], F32, tag="g128")
            nc.sync.dma_start(out=g128[:, :], in_=g128_d[:, :])
            bvP = constp.tile([128, 2], F32, tag="bvP")
            nc.sync.dma_start(out=bvP[:, :], in_=bvP_d[:, :])
            # gb[c] = gamma * bv[c]  (folded v-bias: out += gamma*bv[c])
            gb = constp.tile([128, 2], F32, tag="gb")
            nc.vector.tensor_scalar_mul(gb[:, :], bvP[:, :], g128[:, :])

            ones_cb = constp.tile([128, 1], FP8 if USE_FP8 else BF16,
                                  tag="ones_cb")
            nc.vector.memset(ones_cb[:, :], 1.0)
            shift = constp.tile([128, 1], F32, tag="shift")
            nc.vector.memset(shift[:, :], -EXP_SHIFT)
            ones_row = constp.tile([1, 128], F32, tag="ones_row")
            nc.vector.memset(ones_row[:, :], 1.0)

            # ---- x loads: fp32 via parallel HWDGE (chunked so casts and
            # compute start early), then engine-side casts to bf16 ----
            xf_f = xbp.tile([128, 2, N], F32, tag="xff")
            xq_f = xqfp.tile([128, 2, NH], F32, tag="xqf")
            x_b = xbp.tile([128, 2, N], BF16, tag="xb")
            xq_b = xbp.tile([128, 2, NH], BF16, tag="xqb")
            # chunk-interleaved across halves so the first q/k matmuls
            # (which need both 128-partition halves of column chunk 0)
            # unblock as early as possible
            for c2 in range(2):
                for h in range(2):
                    nc.sync.dma_start(
                        out=xq_f[:, h, c2 * 1024:(c2 + 1) * 1024],
                        in_=xq_d[h * 128:(h + 1) * 128,
                                 c2 * 1024:(c2 + 1) * 1024])
                    nc.scalar.copy(
                        xq_b[:, h, c2 * 1024:(c2 + 1) * 1024],
                        xq_f[:, h, c2 * 1024:(c2 + 1) * 1024])
            for c4 in range(4):
                for h in range(2):
                    nc.sync.dma_start(
                        out=xf_f[:, h, c4 * 1024:(c4 + 1) * 1024],
                        in_=xf_d[h * 128:(h + 1) * 128,
                                 c4 * 1024:(c4 + 1) * 1024])
                    nc.vector.tensor_copy(
                        x_b[:, h, c4 * 1024:(c4 + 1) * 1024],
                        xf_f[:, h, c4 * 1024:(c4 + 1) * 1024])

            # ---- projections ----
            # q (4x-replicated rows): [128, NH]
            q_sb = projp.tile([128, NH], BF16, tag="q")
            for ic in range(NH // 512):
                ps = psB.tile([128, 512], F32, tag="psb")
                nc.tensor.matmul(
                    ps[:, :], lhsT=w_b[:, 0, 0:128],
                    rhs=xq_b[:, 0, ic * 512:(ic + 1) * 512],
                    start=True, stop=False)
                nc.tensor.matmul(
                    ps[:, :], lhsT=w_b[:, 1, 0:128],
                    rhs=xq_b[:, 1, ic * 512:(ic + 1) * 512],
                    start=False, stop=True)
                nc.vector.tensor_scalar_add(
                    q_sb[:, ic * 512:(ic + 1) * 512], ps[:, :], bq4[:, :])

            # k (4x-replicated rows): [128, N]
            k_sb = projp.tile([128, N], BF16, tag="k")
            for ic in range(N // 512):
                ps = psB.tile([128, 512], F32, tag="psb")
                nc.tensor.matmul(
                    ps[:, :], lhsT=w_b[:, 0, 128:256],
                    rhs=x_b[:, 0, ic * 512:(ic + 1) * 512],
                    start=True, stop=False)
                nc.tensor.matmul(
                    ps[:, :], lhsT=w_b[:, 1, 128:256],
                    rhs=x_b[:, 1, ic * 512:(ic + 1) * 512],
                    start=False, stop=True)
                nc.vector.tensor_scalar_add(
                    k_sb[:, ic * 512:(ic + 1) * 512], ps[:, :], bk4[:, :])

            # v^T tiles: [j-tile partitions, c]; bv folded into the epilogue
            v_sb = projp.tile([128, JT, C], FP8 if USE_FP8 else BF16, tag="v")
            for nt in range(JT):
                ps = psB.tile([128, 512], F32, tag="psb")
                nc.tensor.matmul(
                    ps[:, 0:C], lhsT=x_b[:, 0, nt * 128:(nt + 1) * 128],
                    rhs=w_b[:, 0, 256:512], start=True, stop=False)
                nc.tensor.matmul(
                    ps[:, 0:C], lhsT=x_b[:, 1, nt * 128:(nt + 1) * 128],
                    rhs=w_b[:, 1, 256:512], start=False, stop=True)
                nc.vector.tensor_copy(v_sb[:, nt, :], ps[:, 0:C])

            # ---- attention blocks: 4 i-blocks of 512 tokens ----
            for b in range(NBLK):
                i0 = b * 512

                e_blk = eblkp.tile([128, JT, 512], FP8 if USE_FP8 else BF16,
                                   tag="e")
                rs_ps = psR.tile([128, 512], F32, tag="psr")

                # phase A: scores (2x row-packed K=32 matmuls, double-
                # buffered PSUM so exp(g) overlaps scores(g+1)) + exp;
                # rowsum[i] accumulates on PE via ones^T @ e (M=1 matmuls)
                for g in range(JT // 2):
                    ps = psA.tile([128, 2, 512], F32, tag="psa")
                    for t in range(2):
                        jt = 2 * g + t
                        nc.tensor.matmul(
                            ps[:, t, :],
                            lhsT=k_sb[32 * t:32 * (t + 1),
                                      jt * 128:(jt + 1) * 128],
                            rhs=q_sb[32 * t:32 * (t + 1), i0:i0 + 512],
                            start=True, stop=True,
                            tile_position=(32 * t, 0))
                    if USE_FP8:
                        nc.scalar.activation(
                            e_blk[:, 2 * g:2 * g + 2, :], ps[:, :, :],
                            act.Exp, bias=shift[:, :])
                    else:
                        nc.scalar.activation(
                            e_blk[:, 2 * g:2 * g + 2, :], ps[:, :, :],
                            act.Exp)
                    for t in range(2):
                        jt = 2 * g + t
                        nc.tensor.matmul(
                            rs_ps[0:1, :], lhsT=ones_cb[:, :],
                            rhs=e_blk[:, jt, :],
                            start=(jt == 0), stop=(jt == JT - 1))

                # rowsum -> broadcast to 128 partitions -> 1/rs -> *gamma
                rs_sb = smallp.tile([1, 512], F32, tag="rs")
                nc.vector.tensor_copy(rs_sb[:, :], rs_ps[0:1, :])
                bc_ps = psR.tile([128, 512], F32, tag="psr")
                nc.tensor.matmul(
                    bc_ps[:, :], lhsT=ones_row[:, :], rhs=rs_sb[:, :],
                    start=True, stop=True)
                inv = smallp.tile([128, 512], F32, tag="inv")
                nc.vector.reciprocal(inv[:, :], bc_ps[:, :])
                rg = smallp.tile([128, 512], F32, tag="rg")
                nc.vector.tensor_scalar_mul(rg[:, :], inv[:, :], g128[:, :])

                # phase B: out[c, i] accumulation over j, then epilogue:
                # res = (acc * gamma/rowsum) + gamma*bv[c] + x
                for ch in range(2):
                    acc = psB.tile([128, 512], F32, tag="psb")
                    if USE_FP8:
                        for jp in range(JT // 2):
                            nc.tensor.matmul(
                                acc[:, :],
                                lhsT=v_sb[:, 2 * jp:2 * jp + 2,
                                          ch * 128:(ch + 1) * 128],
                                rhs=e_blk[:, 2 * jp:2 * jp + 2, :],
                                perf_mode=mybir.MatmulPerfMode.DoubleRow,
                                start=(jp == 0), stop=(jp == JT // 2 - 1))
                    else:
                        for jt in range(JT):
                            nc.tensor.matmul(
                                acc[:, :],
                                lhsT=v_sb[:, jt, ch * 128:(ch + 1) * 128],
                                rhs=e_blk[:, jt, :],
                                start=(jt == 0), stop=(jt == JT - 1))
                    res = resp.tile([128, 512], F32, tag="res")
                    nc.vector.tensor_mul(res[:, :], acc[:, :], rg[:, :])
                    nc.vector.scalar_tensor_tensor(
                        res[:, :], res[:, :], gb[:, ch:ch + 1],
                        xq_f[:, ch, i0:i0 + 512],
                        op0=mybir.AluOpType.add, op1=mybir.AluOpType.add)
                    nc.sync.dma_start(
                        out=out_d[ch * 128:(ch + 1) * 128, i0:i0 + 512],
                        in_=res[:, :])

    return nc


_CACHE = {}


def _make_in_maps(x, Wq, bq, Wk, bk, Wv, bv, gamma):
    # host-side layout prep (pure relayout, no arithmetic)
    wT = np.concatenate(
        [
            np.tile(np.ascontiguousarray(Wq.T), (1, 4)),
            np.tile(np.ascontiguousarray(Wk.T), (1, 4)),
            np.ascontiguousarray(Wv.T),
        ],
        axis=1,
    ).astype(np.float32)                      # [256, 512]
    bq4 = np.tile(bq, 4).reshape(128, 1).astype(np.float32)
    bk4 = np.tile(bk, 4).reshape(128, 1).astype(np.float32)
    bvP = np.ascontiguousarray(bv.reshape(2, 128).T).astype(np.float32)
    g128 = np.full((128, 1), float(gamma.reshape(-1)[0]), dtype=np.float32)

    core_ids = list(range(8))
    in_maps = []
    for core in core_ids:
        b, half = divmod(core, 2)
        xf = np.ascontiguousarray(x[b].reshape(C, N))
        xq = np.ascontiguousarray(xf[:, half * NH:(half + 1) * NH])
        in_maps.append(
            {
                "xf": xf,
                "xq": xq,
                "wT": wT,
                "bq4": bq4,
                "bk4": bk4,
                "bvP": bvP,
                "g128": g128,
            }
        )
    return in_maps


def kernel(x, Wq, bq, Wk, bk, Wv, bv, gamma):
    x = np.asarray(x, dtype=np.float32)
    Wq = np.asarray(Wq, dtype=np.float32)
    bq = np.asarray(bq, dtype=np.float32)
    Wk = np.asarray(Wk, dtype=np.float32)
    bk = np.asarray(bk, dtype=np.float32)
    Wv = np.asarray(Wv, dtype=np.float32)
    bv = np.asarray(bv, dtype=np.float32)
    gamma = np.asarray(gamma, dtype=np.float32)

    if "nc" not in _CACHE:
        _CACHE["nc"] = _build_program()
    nc = _CACHE["nc"]

    in_maps = _make_in_maps(x, Wq, bq, Wk, bk, Wv, bv, gamma)
    core_ids = list(range(8))

    from concourse.bass_utils import run_bass_kernel_spmd

    res = run_bass_kernel_spmd(nc, in_maps, core_ids)

    out = np.empty((B, C, N), dtype=np.float32)
    for core in core_ids:
        b, half = divmod(core, 2)
        out[b, :, half * NH:(half + 1) * NH] = res.results[core]["out"]
    return out.reshape(B, C, H, W)



# revision 3
# speedup vs baseline: 1.2146x; 1.2146x over previous
"""Position-attention (SAGAN-style) Bass kernel for 8 Trainium2 NeuronCores.

Reference computation (per batch b, with n = H*W = 4096 spatial tokens):
    q = Wq @ x + bq            [32, n]
    k = Wk @ x + bk            [32, n]
    v = Wv @ x + bv            [256, n]
    att = softmax_j(q_i . k_j) [n, n]
    out = gamma * (v @ att^T) + x

Sharding: 8 cores = 4 batches x 2 token-halves; disjoint outputs, no
collectives. SPMD-uniform program: the host permutes each core's x so its
own 2048 tokens are columns [0:2048) (attention is permutation-invariant
over j, and out/q only touch own columns).

Layout/engine choices:
  - scores^T[j, i] = lhsT(k[d, j_tile]) . rhs(q[d, i]): K=32 contraction,
    4-way PE row-tiling (tile_position=(32t, 0)); q/k built 4x-replicated
    across partitions so row group t finds operands at base partition 32t.
  - rowsum[i] = sum_j e^T[j, i]: M=1 ones-matmuls, 4-way PE column-tiling
    (tile_position=(0, 32c)) -> 4 partials at partitions {0,32,64,96} of
    one PSUM bank. A leading zero-matmul (lhsT=0, M=128) opens the bank:
    one whole-bank has_written clear + zeroes garbage partitions, so the
    4 col-groups can all accumulate with start=False and an all-ones
    K=128 matmul later does combine+broadcast in one shot.
  - 1/rowsum via exp(-ln(rs)) on ScalarE (both funcs live in the
    natural_log_exp_and_others table set -> one ACT_TABLE_LOAD); the DVE
    reciprocal is ~6.3 cyc/elem and was 13.4us of Vector time.
  - out[c, i] = sum_j v^T[j, c] e^T[j, i]: K=128 bf16 matmuls accumulated
    over 32 j-tiles in PSUM (fp8 was evaluated and rejected: quantizing
    e/v to any fp8 format costs 4-7e-2 rel err vs the 2e-2 budget).
  - x loaded once (4MB fp32), split across both HWDGE rings (sync +
    scalar queues), own-block0 chunk first so q/scores start early; bf16
    casts chase the DMAs. No separate xq load (was +2MB).
  - softmax needs no max-subtraction: max score ~25 << 88 (fp32 exp
    overflow), e in bf16. exp(-ln(rs + 1e-30)) keeps gamma=0 exact and
    degenerate rows finite.
Matmul operands are bf16 (fp32 PSUM accumulation); the residual `+ x` is
added in exact fp32.
"""

import os
import sys

for _p in (
    "/root/.axon_site",
    "/root/.axon_site/_ro/trn_rl_repo",
    "/root/.axon_site/_ro/pypackages",
    "/opt/trn_rl_repo",
):
    if os.path.isdir(_p) and _p not in sys.path:
        sys.path.append(_p)

import json

import numpy as np

from concourse import bass, mybir
from concourse.tile import TileContext

F32 = mybir.dt.float32
BF16 = mybir.dt.bfloat16

B, C, H, W = 4, 256, 64, 64
N = H * W            # 4096 tokens
NH = N // 2          # 2048 tokens per core (token half)
MID = C // 8         # 32 qk channels
JT = N // 128        # 32 j-tiles of 128 tokens
NBLK = NH // 512     # 4 i-blocks of 512 tokens per core

SCORE_TP = 4         # score matmul row-tiling ways (4 -> positions 0/32/64/96)
RS_COLS = 4          # rowsum matmul column-tiling ways


def _split_multi_waits(bir_bytes: bytes) -> bytes:
    """Workaround for this container's walrus: it accepts at most ONE sem-wait
    command per lowered instruction ('Too many sync wait commands'), while
    bass/Tile freely attach several. Split extra waits onto preceding NoOps
    on the same engine — per-engine program order makes this semantics-
    preserving (all waits still satisfied before the instruction runs)."""
    d = json.loads(bir_bytes)
    n_split = 0
    for f in d.get("functions", []):
        for bb in f.get("blocks", []):
            out = []
            for ins in bb.get("instructions", []):
                si = ins.get("sync_info")
                waits = si.get("on_wait") if si else None
                if waits and len(waits) > 1:
                    for w in waits[:-1]:
                        n_split += 1
                        out.append(
                            {
                                "debug": ins.get("debug", 0),
                                "engine": ins["engine"],
                                "ins": [],
                                "outs": [],
                                "name": f"{ins['name']}-ws{n_split}",
                                "opcode": "NoOp",
                                "sync_info": {"on_wait": [w], "on_update": []},
                            }
                        )
                    si["on_wait"] = [waits[-1]]
                out.append(ins)
            bb["instructions"] = out
    return json.dumps(d).encode()


_ws_applied = False


def _apply_wait_split_patch():
    global _ws_applied
    if _ws_applied:
        return
    _ws_applied = True
    from concourse import bass_utils, bass2jax

    orig = bass_utils.compile_bir_kernel

    def patched(bir_json, tmpdir, neff_name="file.neff"):
        return orig(_split_multi_waits(bytes(bir_json)), tmpdir, neff_name)

    bass_utils.compile_bir_kernel = patched
    bass2jax.compile_bir_kernel = patched


_apply_wait_split_patch()


def _build_program():
    nc = bass.Bass()

    xf_d = nc.declare_dram_parameter("xf", [C, N], F32, isOutput=False)
    wT_d = nc.declare_dram_parameter("wT", [C, 512], F32, isOutput=False)
    bq4_d = nc.declare_dram_parameter("bq4", [128, 1], F32, isOutput=False)
    bk4_d = nc.declare_dram_parameter("bk4", [128, 1], F32, isOutput=False)
    bvP_d = nc.declare_dram_parameter("bvP", [128, 2], F32, isOutput=False)
    g128_d = nc.declare_dram_parameter("g128", [128, 1], F32, isOutput=False)
    out_d = nc.declare_dram_parameter("out", [C, NH], F32, isOutput=True)

    act = mybir.ActivationFunctionType
    add = mybir.AluOpType.add

    with TileContext(nc) as tc:
        with (
            tc.tile_pool(name="const", bufs=1) as constp,
            tc.tile_pool(name="xf", bufs=1) as xfp,
            tc.tile_pool(name="xb", bufs=1) as xbp,
            tc.tile_pool(name="proj", bufs=1) as projp,
            tc.tile_pool(name="eblk", bufs=2) as eblkp,
            tc.tile_pool(name="small", bufs=2) as smallp,
            tc.tile_pool(name="res", bufs=4) as resp,
            tc.tile_pool(name="psA", bufs=1, space="PSUM") as psA,
            tc.tile_pool(name="psB", bufs=2, space="PSUM") as psB,
            tc.tile_pool(name="psR", bufs=2, space="PSUM") as psR,
        ):
            # ---- constants / weights (scalar HWDGE ring) ----
            w_f = constp.tile([128, 2, 512], F32, tag="wf")
            nc.scalar.dma_start(
                out=w_f[:, :, :],
                in_=wT_d[:, :].rearrange("(two p) n -> p two n", two=2))
            bq4 = constp.tile([128, 1], F32, tag="bq4")
            nc.scalar.dma_start(out=bq4[:, :], in_=bq4_d[:, :])
            bk4 = constp.tile([128, 1], F32, tag="bk4")
            nc.scalar.dma_start(out=bk4[:, :], in_=bk4_d[:, :])
            g128 = constp.tile([128, 1], F32, tag="g128")
            nc.scalar.dma_start(out=g128[:, :], in_=g128_d[:, :])
            bvP = constp.tile([128, 2], F32, tag="bvP")
            nc.scalar.dma_start(out=bvP[:, :], in_=bvP_d[:, :])

            w_b = constp.tile([128, 2, 512], BF16, tag="wb")
            nc.vector.tensor_copy(w_b[:, :, :], w_f[:, :, :])
            # gb[c] = gamma * bv[c]  (folded v-bias: out += gamma*bv[c])
            gb = constp.tile([128, 2], F32, tag="gb")
            nc.vector.tensor_scalar_mul(gb[:, :], bvP[:, :], g128[:, :])

            ones_b = constp.tile([128, 1], BF16, tag="ones_b")
            nc.vector.memset(ones_b[:, :], 1.0)
            ones_f = constp.tile([128, 128], F32, tag="ones_f")
            nc.vector.memset(ones_f[:, :], 1.0)
            zero_b = constp.tile([128, 128], BF16, tag="zero_b")
            nc.vector.memset(zero_b[:, :], 0.0)
            eps128 = constp.tile([128, 1], F32, tag="eps")
            nc.vector.memset(eps128[:, :], 1e-30)

            # ---- x loads: own-block0 small chunk first (unblocks q and the
            # first score group), own-rest on the sync ring, other half on
            # the scalar ring so the two HWDGE rings transfer in parallel ----
            xf_f = xfp.tile([128, 2, N], F32, tag="xff")
            x_b = xbp.tile([128, 2, N], BF16, tag="xb")
            pieces = [  # (c0, c1, engine)
                (0, 512, nc.sync),
                (512, 2048, nc.sync),
                (2048, 3072, nc.scalar),
                (3072, 4096, nc.scalar),
            ]
            for c0, c1, eng in pieces:
                eng.dma_start(
                    out=xf_f[:, :, c0:c1],
                    in_=xf_d[:, c0:c1].rearrange("(two p) n -> p two n",
                                                 two=2))
            for c0, c1, _ in pieces:
                for s0 in range(c0, c1, 512):
                    nc.vector.tensor_copy(x_b[:, :, s0:s0 + 512],
                                          xf_f[:, :, s0:s0 + 512])

            # ---- projections ----
            # q (4x-replicated rows): [128, NH]; own columns only
            q_sb = projp.tile([128, NH], BF16, tag="q")
            for ic in range(NH // 512):
                ps = psB.tile([128, 512], F32, tag="psb")
                nc.tensor.matmul(
                    ps[:, :], lhsT=w_b[:, 0, 0:128],
                    rhs=x_b[:, 0, ic * 512:(ic + 1) * 512],
                    start=True, stop=False)
                nc.tensor.matmul(
                    ps[:, :], lhsT=w_b[:, 1, 0:128],
                    rhs=x_b[:, 1, ic * 512:(ic + 1) * 512],
                    start=False, stop=True)
                nc.vector.tensor_scalar_add(
                    q_sb[:, ic * 512:(ic + 1) * 512], ps[:, :], bq4[:, :])

            # k (4x-replicated rows): [128, N]
            k_sb = projp.tile([128, N], BF16, tag="k")
            for ic in range(N // 512):
                ps = psB.tile([128, 512], F32, tag="psb")
                nc.tensor.matmul(
                    ps[:, :], lhsT=w_b[:, 0, 128:256],
                    rhs=x_b[:, 0, ic * 512:(ic + 1) * 512],
                    start=True, stop=False)
                nc.tensor.matmul(
                    ps[:, :], lhsT=w_b[:, 1, 128:256],
                    rhs=x_b[:, 1, ic * 512:(ic + 1) * 512],
                    start=False, stop=True)
                nc.vector.tensor_scalar_add(
                    k_sb[:, ic * 512:(ic + 1) * 512], ps[:, :], bk4[:, :])

            # v^T tiles, flat [128, JT*256]: tile jt at cols [jt*256, +256);
            # two tiles share one PSUM bank so the evacuating cast is a
            # single [128, 512] copy. bv folds into the epilogue.
            v_sb = projp.tile([128, JT * C], BF16, tag="v")
            for p in range(JT // 2):
                ps = psB.tile([128, 512], F32, tag="psb")
                for t in range(2):
                    jt = 2 * p + t
                    nc.tensor.matmul(
                        ps[:, t * 256:t * 256 + C],
                        lhsT=x_b[:, 0, jt * 128:(jt + 1) * 128],
                        rhs=w_b[:, 0, 256:512], start=True, stop=False)
                    nc.tensor.matmul(
                        ps[:, t * 256:t * 256 + C],
                        lhsT=x_b[:, 1, jt * 128:(jt + 1) * 128],
                        rhs=w_b[:, 1, 256:512], start=False, stop=True)
                nc.vector.tensor_copy(v_sb[:, p * 512:(p + 1) * 512],
                                      ps[:, :])

            # ---- attention blocks: 4 i-blocks of 512 tokens ----
            for b in range(NBLK):
                i0 = b * 512

                e_blk = eblkp.tile([128, JT, 512], BF16, tag="e")
                rs_ps = psR.tile([128, 512], F32, tag="psr")
                # bank-opener: one whole-bank has_written clear + zeroes, so
                # the col-tiled partials below all run start=False and the
                # combine matmul can sum all 128 partitions.
                nc.tensor.matmul(
                    rs_ps[:, :], lhsT=zero_b[:, :], rhs=w_b[:, 0, :],
                    start=True, stop=False, skip_group_check=True)

                # phase A: 4-way row-tiled scores + exp, then 4-way
                # col-tiled rowsum accumulation
                for g in range(JT // SCORE_TP):
                    ps4 = psA.tile([128, SCORE_TP, 512], F32, tag="psa")
                    for t in range(SCORE_TP):
                        jt = SCORE_TP * g + t
                        nc.tensor.matmul(
                            ps4[:, t, :],
                            lhsT=k_sb[32 * t:32 * (t + 1),
                                      jt * 128:(jt + 1) * 128],
                            rhs=q_sb[32 * t:32 * (t + 1), i0:i0 + 512],
                            start=True, stop=True,
                            tile_position=(32 * t, 0))
                    nc.scalar.activation(
                        e_blk[:, SCORE_TP * g:SCORE_TP * (g + 1), :],
                        ps4[:, :, :], act.Exp)
                    for t in range(SCORE_TP):
                        jt = SCORE_TP * g + t
                        c = jt % RS_COLS
                        nc.tensor.matmul(
                            rs_ps[32 * c:32 * c + 1, :],
                            lhsT=ones_b[:, :], rhs=e_blk[:, jt, :],
                            start=False, stop=(jt >= JT - RS_COLS),
                            tile_position=(0, 32 * c),
                            skip_group_check=True)

                # rowsum partials -> SBUF; all-ones K=128 matmul does
                # combine + broadcast to 128 partitions in one shot;
                # 1/rs as exp(-ln(rs)) on ScalarE; * gamma on DVE
                rs_sb = smallp.tile([128, 512], F32, tag="rs")
                nc.vector.tensor_copy(rs_sb[:, :], rs_ps[:, :])
                bc_ps = psR.tile([128, 512], F32, tag="psr")
                nc.tensor.matmul(
                    bc_ps[:, :], lhsT=ones_f[:, :], rhs=rs_sb[:, :],
                    start=True, stop=True)
                ln_sb = smallp.tile([128, 512], F32, tag="ln")
                nc.scalar.activation(ln_sb[:, :], bc_ps[:, :], act.Ln,
                                     bias=eps128[:, :])
                inv_sb = smallp.tile([128, 512], F32, tag="inv")
                nc.scalar.activation(inv_sb[:, :], ln_sb[:, :], act.Exp,
                                     scale=-1.0)
                rg = smallp.tile([128, 512], F32, tag="rg")
                nc.vector.tensor_scalar_mul(rg[:, :], inv_sb[:, :],
                                            g128[:, :])

                # phase B: out[c, i] accumulation over j, then epilogue:
                # res = (acc * gamma/rowsum) + gamma*bv[c] + x
                for ch in range(2):
                    acc = psB.tile([128, 512], F32, tag="psb")
                    for jt in range(JT):
                        nc.tensor.matmul(
                            acc[:, :],
                            lhsT=v_sb[:, jt * 256 + ch * 128:
                                      jt * 256 + ch * 128 + 128],
                            rhs=e_blk[:, jt, :],
                            start=(jt == 0), stop=(jt == JT - 1))
                    res = resp.tile([128, 512], F32, tag="res")
                    nc.vector.tensor_mul(res[:, :], acc[:, :], rg[:, :])
                    nc.vector.scalar_tensor_tensor(
                        res[:, :], res[:, :], gb[:, ch:ch + 1],
                        xf_f[:, ch, i0:i0 + 512],
                        op0=add, op1=add)
                    nc.sync.dma_start(
                        out=out_d[ch * 128:(ch + 1) * 128, i0:i0 + 512],
                        in_=res[:, :])

    return nc


_CACHE = {}


def _make_in_maps(x, Wq, bq, Wk, bk, Wv, bv, gamma):
    # host-side layout prep (pure relayout, no arithmetic)
    wT = np.concatenate(
        [
            np.tile(np.ascontiguousarray(Wq.T), (1, 4)),
            np.tile(np.ascontiguousarray(Wk.T), (1, 4)),
            np.ascontiguousarray(Wv.T),
        ],
        axis=1,
    ).astype(np.float32)                      # [256, 512]
    bq4 = np.tile(bq, 4).reshape(128, 1).astype(np.float32)
    bk4 = np.tile(bk, 4).reshape(128, 1).astype(np.float32)
    bvP = np.ascontiguousarray(bv.reshape(2, 128).T).astype(np.float32)
    g128 = np.full((128, 1), float(gamma.reshape(-1)[0]), dtype=np.float32)

    in_maps = []
    for core in range(8):
        b, half = divmod(core, 2)
        xf = x[b].reshape(C, N)
        # rotate so this core's own half-columns come first: the program
        # is SPMD-uniform (own tokens = columns [0:2048)); attention is
        # permutation-invariant over j
        xp = np.ascontiguousarray(
            np.concatenate([xf[:, half * NH:(half + 1) * NH],
                            xf[:, (1 - half) * NH:(2 - half) * NH]], axis=1))
        in_maps.append(
            {
                "xf": xp,
                "wT": wT,
                "bq4": bq4,
                "bk4": bk4,
                "bvP": bvP,
                "g128": g128,
            }
        )
    return in_maps


def kernel(x, Wq, bq, Wk, bk, Wv, bv, gamma):
    x = np.asarray(x, dtype=np.float32)
    Wq = np.asarray(Wq, dtype=np.float32)
    bq = np.asarray(bq, dtype=np.float32)
    Wk = np.asarray(Wk, dtype=np.float32)
    bk = np.asarray(bk, dtype=np.float32)
    Wv = np.asarray(Wv, dtype=np.float32)
    bv = np.asarray(bv, dtype=np.float32)
    gamma = np.asarray(gamma, dtype=np.float32)

    if "nc" not in _CACHE:
        _CACHE["nc"] = _build_program()
    nc = _CACHE["nc"]

    in_maps = _make_in_maps(x, Wq, bq, Wk, bk, Wv, bv, gamma)
    core_ids = list(range(8))

    from concourse.bass_utils import run_bass_kernel_spmd

    res = run_bass_kernel_spmd(nc, in_maps, core_ids)

    out = np.empty((B, C, N), dtype=np.float32)
    for core in core_ids:
        b, half = divmod(core, 2)
        out[b, :, half * NH:(half + 1) * NH] = res.results[core]["out"]
    return out.reshape(B, C, H, W)
